# revision 54
# baseline (speedup 1.0000x reference)
"""Trainium2 Bass kernel for the LoRA-BC block (nn_LoRABCBlock) — v2.

Reference computation:
    base = x @ w_base.T
    h = layernorm(x) * gamma + beta
    qkv = h @ w_qkv.T ; attention (2 heads, head_dim 32) over full sequence
    attn_out = ao @ w_attn_out.T
    delta = ((h + attn_out) @ lora_down) @ lora_up
    out = base + (1/8) * delta

Key algebraic restructurings (all exact):
  * attn_out only feeds the LoRA path:
        t := (h + attn_out) @ lora_down = ldg.T @ zh + W2.T @ ao + ld.T @ beta
    with W2 = w_attn_out.T @ lora_down ([64, 8], host-precomputed), so the
    full attn_out projection (and its 1024-dim residual) never exists.
  * LayerNorm affine is folded into the consumers: only zh = (x - mu) * rstd
    is materialized (transposed).  gamma folds into w_qkv / lora_down rows
    (host); beta becomes per-output bias columns / rank-1 matmul terms.
  * base = x @ w_base.T is rebuilt from zh:
        base[m, :] = std[m] * (zh @ w_base.T)[m, :] + mu[m] * wsum[:]
    std scaling rides the PSUM evacuation (per-partition scalar); the
    mu (x) wsum rank-1 term rides the LoRA-up matmul as a 9th contraction
    row ([t; mu] @ [lora_up * s; wsum]).
  * Scores are computed TRANSPOSED (sT[j, m] = k_j . q_m) so exp(sT) is
    directly the stationary operand of the ao matmul — zero probability
    transposes.  Softmax normalization is deferred: v gets 8 ones-columns
    so the unnormalized ao matmul also yields replicated row-sums, and the
    reciprocal is applied after the tiny W2 projection ([8, m] instead of
    [1024, m]).

Sharding: data-parallel over (batch, seq-half) -> 8 cores, as v1.
All weights host-pre-transposed/folded/cast to bf16.  x is host-cast to
bf16 (stats + matmuls in bf16, fp32 accumulation everywhere).
"""

import sys

sys.path.insert(0, "/opt/trn_rl_repo")

from contextlib import ExitStack

import numpy as np
import ml_dtypes

import concourse.bass as bass
import concourse.tile as tile
from concourse import bacc, mybir
from concourse.bass_utils import run_bass_kernel_spmd
from concourse.masks import make_identity

F32 = mybir.dt.float32
BF16 = mybir.dt.bfloat16
AF = mybir.ActivationFunctionType
ALU = mybir.AluOpType
BF = ml_dtypes.bfloat16

E = 1024          # embed dim
DM = 1024         # d_model
R = 8             # lora rank
SCALING = 1.0 / R
DA = 64           # attn dim
NH = 2            # heads
HD = DA // NH     # head dim = 32
SOWN = 1024       # rows owned per core
SFULL = 2048      # rows per batch element
NC = 8            # cores
P = 128
KT = E // P       # 8 k-tiles
MT = SOWN // P    # 8 own m-tiles
ST = SFULL // P   # 16 sequence tiles
ATT_SCALE = float(HD) ** -0.5
VE = HD + R       # v_ext cols per head: 32 v + 8 ones


def build_kernel():
    nc = bacc.Bacc("TRN2", target_bir_lowering=False, debug=False, num_devices=NC)

    x_own = nc.dram_tensor("x_own", [SOWN, E], BF16, kind="ExternalInput").ap()
    x_oth = nc.dram_tensor("x_oth", [SOWN, E], BF16, kind="ExternalInput").ap()
    wbT_d = nc.dram_tensor("wbT", [E, DM], BF16, kind="ExternalInput").ap()
    wqkvT_d = nc.dram_tensor("wqkvT", [E, 3 * DA], BF16, kind="ExternalInput").ap()
    ldg_d = nc.dram_tensor("ldg", [E, R], BF16, kind="ExternalInput").ap()
    lux_d = nc.dram_tensor("lu_ext", [33, DM], BF16, kind="ExternalInput").ap()
    w2_d = nc.dram_tensor("w2", [HD, NH, R], BF16, kind="ExternalInput").ap()
    qkb_d = nc.dram_tensor("qk_bias", [2 * DA, 1], F32, kind="ExternalInput").ap()
    vb_d = nc.dram_tensor("v_bias", [DA, 1], F32, kind="ExternalInput").ap()
    tb_d = nc.dram_tensor("t_bias", [R, 1], F32, kind="ExternalInput").ap()
    out_d = nc.dram_tensor("out", [SOWN, DM], F32, kind="ExternalOutput").ap()

    with tile.TileContext(nc) as tc, ExitStack() as ctx:
        persist = ctx.enter_context(tc.tile_pool(name="persist", bufs=1))
        ld_pool = ctx.enter_context(tc.tile_pool(name="loads", bufs=5))
        zh_pool = ctx.enter_context(tc.tile_pool(name="zh", bufs=3))
        st_pool = ctx.enter_context(tc.tile_pool(name="stats", bufs=4))
        asm_pool = ctx.enter_context(tc.tile_pool(name="asm", bufs=3))
        ps = ctx.enter_context(tc.tile_pool(name="ps", bufs=1, space="PSUM"))

        _psn = [0]

        def ps_tile(shape, dtype, tag, bufs):
            _psn[0] += 1
            return ps.tile(shape, dtype, tag=tag, bufs=bufs,
                           name=f"ps_{tag}_{_psn[0]}")

        # ---------------- x loads first, then constants + weights ----------
        # x-tile DMAs are issued for all 16 tiles up front (gpsimd queue,
        # pool rotation throttles to 5 in flight) so the wire is busy from
        # instruction 0; weights ride the sync queue in parallel.
        xfs = []
        for st in range(ST):
            own = st < MT
            src = x_own if own else x_oth
            row0 = st * P if own else (st - MT) * P
            xf = ld_pool.tile([P, E], BF16, tag="xin", name=f"xf{st}")
            nc.gpsimd.dma_start(out=xf, in_=src[row0:row0 + P, :])
            xfs.append(xf)

        # NOTE: contraction index k is laid out k = kt*128 + p; weights use
        # the same split.
        wqkvT = persist.tile([P, KT, 3 * DA], BF16, tag="wqkvT")
        nc.sync.dma_start(out=wqkvT,
                          in_=wqkvT_d.rearrange("(kt p) a -> p kt a", p=P))
        wbT = persist.tile([P, KT, DM], BF16, tag="wbT")
        nc.sync.dma_start(out=wbT, in_=wbT_d.rearrange("(kt p) n -> p kt n", p=P))
        ldg = persist.tile([P, KT, R], BF16, tag="ldg")
        nc.sync.dma_start(out=ldg, in_=ldg_d.rearrange("(kt p) r -> p kt r", p=P))
        lux = persist.tile([33, DM], BF16, tag="lux")
        nc.sync.dma_start(out=lux, in_=lux_d)
        w2 = persist.tile([HD, NH, R], BF16, tag="w2")
        nc.sync.dma_start(out=w2, in_=w2_d)
        qkb = persist.tile([2 * DA, 1], F32, tag="qkb")
        nc.sync.dma_start(out=qkb, in_=qkb_d)
        vb = persist.tile([DA, 1], F32, tag="vb")
        nc.sync.dma_start(out=vb, in_=vb_d)
        tb = persist.tile([R, 1], F32, tag="tb")
        nc.sync.dma_start(out=tb, in_=tb_d)

        ident = persist.tile([P, P], BF16, tag="ident")
        make_identity(nc, ident)
        eps_t = persist.tile([P, 1], F32, tag="eps")
        nc.vector.memset(eps_t, 1e-5)

        # ---------------- persistent activations ----------------
        zhT = persist.tile([P, KT, ST, P], BF16, tag="zhT")
        expT = persist.tile([P, ST, NH, SOWN], BF16, tag="expT")
        # q/k live twice (rows 0:64 and 64:128) so the four score matmuls of
        # a j-tile occupy all four 32-row PE groups and run concurrently
        qT = persist.tile([P, SOWN], BF16, tag="qT")
        kTt = persist.tile([P, SFULL], BF16, tag="kTt")
        v_ext = persist.tile([P, ST, NH, VE], BF16, tag="v_ext")
        nc.vector.memset(v_ext[:, :, :, HD:VE], 1.0)
        o_base = persist.tile([P, MT, DM], F32, tag="o_base")
        stds = persist.tile([P, MT], F32, tag="stds")
        # rows 0-7: t; rows 8-31: zero (DVE writes need 32-aligned partition
        # starts, so mu lives at row 32 and lu_ext rows 8-31 are zero)
        tT_ext = persist.tile([33, SOWN], BF16, tag="tT_ext")
        nc.vector.memset(tT_ext, 0.0)
        aoU = persist.tile([VE, NH, SOWN], BF16, tag="aoU")
        recip8 = persist.tile([R, NH, SOWN], F32, tag="recip8")

        # ---------------- phase 1: stats + zh + transposes + base ----------
        for st in range(ST):
            own = st < MT
            xf = xfs[st]

            stats = st_pool.tile([P, 2, 6], F32, tag="bnstats")
            xr = xf.rearrange("p (n f) -> p n f", f=512)
            for sg in range(2):
                nc.vector.bn_stats(out=stats[:, sg, :], in_=xr[:, sg, :])
            mv = st_pool.tile([P, 2], F32, tag="mv")
            nc.vector.bn_aggr(out=mv, in_=stats)

            # std = sqrt(var + eps)
            std_dst = stds[:, st:st + 1] if own else st_pool.tile([P, 1], F32,
                                                                  tag="stdo")
            nc.scalar.activation(out=std_dst, in_=mv[:, 1:2], func=AF.Sqrt,
                                 bias=eps_t)

            # zh = (x - mu) * rstd   (bf16)
            rstd = st_pool.tile([P, 1], F32, tag="rstd")
            nc.vector.reciprocal(out=rstd, in_=std_dst)
            nmr = st_pool.tile([P, 1], F32, tag="nmr")
            nc.vector.tensor_scalar(out=nmr, in0=mv[:, 0:1], scalar1=rstd,
                                    scalar2=-1.0, op0=ALU.mult, op1=ALU.mult)
            zh = zh_pool.tile([P, E], BF16, tag="zh")
            nc.scalar.activation(out=zh, in_=xf, func=AF.Identity,
                                 scale=rstd, bias=nmr)

            if own:
                # mu row (bf16) -> tT_ext[R, :] via PE transpose
                mv_bf = st_pool.tile([P, 1], BF16, tag="mvbf")
                nc.vector.tensor_copy(out=mv_bf, in_=mv[:, 0:1])
                tpm = ps_tile([P, 512], BF16, "tp", 1)
                nc.tensor.transpose(tpm[0:1, 0:P], mv_bf, ident)
                nc.vector.tensor_copy(out=tT_ext[32:33, st * P:(st + 1) * P],
                                      in_=tpm[0:1, 0:P])

            # 8 transposes of zh -> zhT, grouped 4 per psum tile
            for g in range(2):
                tpt = ps_tile([P, 512], BF16, "tp", 1)
                for kk in range(4):
                    k = g * 4 + kk
                    nc.tensor.transpose(tpt[:, kk * P:(kk + 1) * P],
                                        zh[:, k * P:(k + 1) * P], ident)
                dst = zhT[:, g * 4:(g + 1) * 4, st, :]
                srcv = tpt.rearrange("p (kk c) -> p kk c", c=P)
                if (st + g) % 2 == 0:
                    nc.vector.tensor_copy(out=dst, in_=srcv)
                else:
                    nc.scalar.copy(out=dst, in_=srcv)

            # base matmuls ride inside phase 1 (mt = st-2) so the PE queue
            # always holds ready work while the zh chain paces the loop
            if 2 <= st < 2 + 6:
                mt = st - 2
                for grp in range(2):
                    pb = ps_tile([P, 512], F32, "mm", 3)
                    for k in range(KT):
                        nc.tensor.matmul(pb, zhT[:, k, mt, :],
                                         wbT[:, k, grp * 512:(grp + 1) * 512],
                                         start=(k == 0), stop=(k == KT - 1))
                    osl = o_base[:, mt, grp * 512:(grp + 1) * 512]
                    if grp == 0:
                        nc.scalar.activation(out=osl, in_=pb, func=AF.Copy,
                                             scale=stds[:, mt:mt + 1])
                    else:
                        nc.vector.tensor_scalar_mul(out=osl, in0=pb,
                                                    scalar1=stds[:, mt:mt + 1])

        # ---------------- phase 2: qkv projections ----------------
        # q,k for own rows: psum [128a, 512m] x 2 groups, accumulate over k
        pqk = [ps_tile([P, 512], F32, "mm", 3) for _ in range(2)]
        for k in range(KT):
            for grp in range(2):
                nc.tensor.matmul(pqk[grp], wqkvT[:, k, 0:P],
                                 zhT[:, k, grp * 4:(grp + 1) * 4, :],
                                 start=(k == 0), stop=(k == KT - 1))
        for grp in range(2):
            sl = slice(grp * 512, (grp + 1) * 512)
            nc.vector.tensor_scalar_add(out=qT[0:DA, sl], in0=pqk[grp][0:DA, :],
                                        scalar1=qkb[0:DA])
            # h0 q/k duplicated at rows 64:96 (PE row-group 2; group 3 at 96
            # is a hardware no-go) so grp1's h0 score matmul runs in its own
            # row group concurrently with grp0's
            nc.vector.tensor_scalar_add(out=qT[DA:DA + HD, sl],
                                        in0=pqk[grp][0:HD, :],
                                        scalar1=qkb[0:HD])
            nc.vector.tensor_scalar_add(out=kTt[0:DA, sl], in0=pqk[grp][DA:P, :],
                                        scalar1=qkb[DA:2 * DA])
            nc.vector.tensor_scalar_add(out=kTt[DA:DA + HD, sl],
                                        in0=pqk[grp][DA:DA + HD, :],
                                        scalar1=qkb[DA:DA + HD])
        # k for other rows
        for grp in range(2):
            pko = ps_tile([DA, 512], F32, "mm", 3)
            for k in range(KT):
                nc.tensor.matmul(pko, wqkvT[:, k, DA:P],
                                 zhT[:, k, MT + grp * 4:MT + (grp + 1) * 4, :],
                                 start=(k == 0), stop=(k == KT - 1))
            osl = slice(SOWN + grp * 512, SOWN + (grp + 1) * 512)
            nc.vector.tensor_scalar_add(out=kTt[0:DA, osl], in0=pko,
                                        scalar1=qkb[DA:2 * DA])
            nc.vector.tensor_scalar_add(out=kTt[DA:DA + HD, osl],
                                        in0=pko[0:HD, :],
                                        scalar1=qkb[DA:DA + HD])
        # v: produce transposed vT [64d, 2048j] with a big moving stream
        # (w_v stationary), then 16 cheap PE transposes into natural layout.
        vT_sb = persist.tile([DA, SFULL], BF16, tag="vT_sb")
        for grp in range(4):
            pvT = ps_tile([DA, 512], F32, "mm", 3)
            for k in range(KT):
                nc.tensor.matmul(pvT, wqkvT[:, k, 2 * DA:3 * DA],
                                 zhT[:, k, grp * 4:(grp + 1) * 4, :],
                                 start=(k == 0), stop=(k == KT - 1))
            # beta-fold bias is per-d = per-partition in this layout
            nc.vector.tensor_scalar_add(
                out=vT_sb[:, grp * 512:(grp + 1) * 512], in0=pvT, scalar1=vb)
        for g8 in range(2):
            tpt = ps_tile([P, 512], BF16, "tp", 1)
            for jj in range(8):
                jt = g8 * 8 + jj
                nc.tensor.transpose(tpt[:, jj * DA:(jj + 1) * DA],
                                    vT_sb[:, jt * P:(jt + 1) * P],
                                    ident[0:DA, 0:DA])
            nc.vector.tensor_copy(
                out=v_ext[:, g8 * 8:(g8 + 1) * 8, :, 0:HD],
                in_=tpt.rearrange("p (jj h c) -> p jj h c", jj=8, h=NH))

        # ---------------- phase 3: scores^T + exp + ao^T + base ------------
        # Per j-tile: 4 score matmuls -> per-head exp -> 4 ao-accumulate
        # pairs (heads col-tiled into disjoint PE column groups / psum
        # partition ranges).  Base matmuls interleave on odd j-tiles so the
        # PE queue always holds ready work and the clock stays ramped.
        pao = [ps_tile([P, 512], F32, "mm", 3) for _ in range(2)]
        for jt in range(ST):
            sps = []
            for h in range(NH):
                sp = ps_tile([P, SOWN], F32, "s", 2)
                for grp in range(2):
                    # rows: (h0,g0)@0 (h1,*)@32 (h0,g1)@64 — 3-way concurrent
                    d0 = HD if h == 1 else DA * grp
                    nc.tensor.matmul(
                        sp[:, grp * 512:(grp + 1) * 512],
                        kTt[d0:d0 + HD, jt * P:(jt + 1) * P],
                        qT[d0:d0 + HD, grp * 512:(grp + 1) * 512],
                        start=True, stop=True)
                nc.scalar.activation(out=expT[:, jt, h, :], in_=sp,
                                     func=AF.Exp, scale=ATT_SCALE)
                sps.append(sp)
            for grp in range(2):
                for h in range(NH):
                    b0 = 64 * h
                    nc.tensor.matmul(pao[grp][b0:b0 + VE, :],
                                     v_ext[:, jt, h, :],
                                     expT[:, jt, h, grp * 512:(grp + 1) * 512],
                                     start=(jt == 0), stop=(jt == ST - 1),
                                     tile_position=(0, b0),
                                     skip_group_check=True)
            if jt in (1, 3):
                mt = 6 + jt // 2
                for grp in range(2):
                    pb = ps_tile([P, 512], F32, "mm", 3)
                    for k in range(KT):
                        nc.tensor.matmul(pb, zhT[:, k, mt, :],
                                         wbT[:, k, grp * 512:(grp + 1) * 512],
                                         start=(k == 0), stop=(k == KT - 1))
                    nc.vector.tensor_scalar_mul(
                        out=o_base[:, mt, grp * 512:(grp + 1) * 512],
                        in0=pb, scalar1=stds[:, mt:mt + 1])

        # ---------------- phase 4: ao evac, W2, t assembly, lora-up, out ---
        for grp in range(2):
            sl = slice(grp * 512, (grp + 1) * 512)
            for h in range(NH):
                b0 = 64 * h
                nc.scalar.copy(out=aoU[:, h, sl],
                               in_=pao[grp][b0:b0 + VE, :])
                # stage row-sums to fp32 SBUF, then fast-approx reciprocal
                # (custom DVE ops misread PSUM operands on hardware)
                rsf = asm_pool.tile([R, 512], F32, tag="rsf")
                nc.scalar.copy(out=rsf, in_=pao[grp][b0 + HD:b0 + VE, :])
                nc.vector.reciprocal_approx_fast(out=recip8[:, h, sl],
                                                 in_=rsf)

        for grp in range(2):
            sl = slice(grp * 512, (grp + 1) * 512)
            u = []
            for h in range(NH):
                pt = ps_tile([R, 512], F32, "mm", 3)
                nc.tensor.matmul(pt, w2[:, h, :], aoU[0:HD, h, sl],
                                 start=True, stop=True)
                uh = asm_pool.tile([R, 512], F32, tag="u")
                # softmax normalization via the precomputed reciprocals
                nc.vector.tensor_mul(out=uh, in0=pt, in1=recip8[:, h, sl])
                u.append(uh)
            pth = ps_tile([R, 512], F32, "mm", 3)
            for k in range(KT):
                nc.tensor.matmul(pth, ldg[:, k, :],
                                 zhT[:, k, grp * 4:(grp + 1) * 4, :],
                                 start=(k == 0), stop=(k == KT - 1))
            s2 = asm_pool.tile([R, 512], F32, tag="u")
            nc.vector.scalar_tensor_tensor(out=s2, in0=pth, scalar=tb,
                                           in1=u[0], op0=ALU.add, op1=ALU.add)
            nc.vector.tensor_add(out=tT_ext[0:R, sl], in0=s2, in1=u[1])

        for mt in range(MT):
            for grp in range(2):
                pl = ps_tile([P, 512], F32, "mm", 3)
                nc.tensor.matmul(pl, tT_ext[:, mt * P:(mt + 1) * P],
                                 lux[:, grp * 512:(grp + 1) * 512],
                                 start=True, stop=True)
                osl = o_base[:, mt, grp * 512:(grp + 1) * 512]
                nc.vector.scalar_tensor_tensor(out=osl, in0=pl, scalar=1.0,
                                               in1=osl, op0=ALU.bypass,
                                               op1=ALU.add)
            eng = nc.gpsimd if mt % 2 == 0 else nc.sync
            eng.dma_start(out=out_d[mt * P:(mt + 1) * P, :],
                          in_=o_base[:, mt, :])

    nc.compile()
    return nc


_NC_CACHE = None


def _get_nc():
    global _NC_CACHE
    if _NC_CACHE is None:
        _NC_CACHE = build_kernel()
    return _NC_CACHE


def prep_weights(w_base, ln_gamma, ln_beta, lora_down, lora_up, w_qkv,
                 w_attn_out):
    """Host-side weight folds/layout (all [weights-only], measured off-device)."""
    w_base = np.asarray(w_base, np.float64)
    g = np.asarray(ln_gamma, np.float64)
    b = np.asarray(ln_beta, np.float64)
    ld = np.asarray(lora_down, np.float64)
    lu = np.asarray(lora_up, np.float64)
    wqkv = np.asarray(w_qkv, np.float64)
    wao = np.asarray(w_attn_out, np.float64)

    wbT = np.ascontiguousarray(w_base.T)                       # [E, DM]
    wqkvT = np.ascontiguousarray((wqkv * g[None, :]).T)        # [E, 192]
    ldg = np.ascontiguousarray(ld * g[:, None])                # [E, R]
    lu_ext = np.concatenate([lu * SCALING,
                             np.zeros((24, DM)),
                             w_base.sum(1)[None, :]], 0)       # [33, DM]
    w2 = (wao.T @ ld).reshape(NH, HD, R).transpose(1, 0, 2)    # [HD, NH, R]
    qk_bias = (wqkv[:2 * DA] @ b)[:, None]                     # [128, 1]
    v_bias = (wqkv[2 * DA:] @ b)[:, None]                      # [64, 1]
    t_bias = (ld.T @ b)[:, None]                               # [R, 1]
    return {
        "wbT": wbT.astype(BF), "wqkvT": wqkvT.astype(BF),
        "ldg": ldg.astype(BF), "lu_ext": np.ascontiguousarray(lu_ext).astype(BF),
        "w2": np.ascontiguousarray(w2).astype(BF),
        "qk_bias": qk_bias.astype(np.float32),
        "v_bias": v_bias.astype(np.float32),
        "t_bias": t_bias.astype(np.float32),
    }


def prep_core_inputs(x, wk, core):
    b, half = divmod(core, 2)
    xb = np.asarray(x[b], BF)
    own = np.ascontiguousarray(xb[half * SOWN:(half + 1) * SOWN])
    oth = np.ascontiguousarray(xb[(1 - half) * SOWN:(2 - half) * SOWN])
    return {"x_own": own, "x_oth": oth, **wk}


def kernel(x, w_base, ln_gamma, ln_beta, lora_down, lora_up, w_qkv, w_attn_out,
           _trace=False):
    x = np.asarray(x, np.float32)
    wk = prep_weights(w_base, ln_gamma, ln_beta, lora_down, lora_up, w_qkv,
                      w_attn_out)
    nc = _get_nc()
    in_maps = [prep_core_inputs(x, wk, c) for c in range(NC)]
    res = run_bass_kernel_spmd(nc, in_maps, core_ids=list(range(NC)), trace=_trace)
    B, S = x.shape[0], x.shape[1]
    out = np.empty((B, S, DM), np.float32)
    for c in range(NC):
        b, half = divmod(c, 2)
        out[b, half * SOWN:(half + 1) * SOWN] = res.results[c]["out"]
    if _trace:
        kernel.last_exec_time_ns = res.exec_time_ns
        kernel.last_results = res
    return out
